# revision 12
# baseline (speedup 1.0000x reference)
"""Multi-head attention kernel for Trainium2, 8 NeuronCores.

Problem: B=2, S=2048, D=1024, H=16, Dk=64, fp32.
  qkv = x @ W_qkv + b_qkv ; per-head scaled-dot-product attention with
  key mask; out = attn_out @ W_out + b_out.

Sharding: DP over batch (2) x TP over head groups (4 groups of 4 heads).
Core c -> (b = c // 4, g = c % 4). Each core computes the partial output
  y_partial[b] = attn_out[:, heads(g)] @ W_out[rows(g)]
and the host sums the 4 partials per batch (row-parallel unshard) and
adds b_out.

v2 design (ACT-exp-roofline oriented; measured rel err ~3e-3):
  * Host ships x^T (and the weights) pre-transposed in bf16, so the
    kernel has no PE transposes and half the x DMA traffic. All matmuls
    run bf16 (1 cycle/row, same rate as f32r) with fp32 PSUM.
  * Q^T/K^T are stored per head, zero-padded to 128 contraction rows
    (head 2m+a keeps its natural partitions a*64..a*64+64; the pad rows
    are zeroed for free by the same DVE write via per-partition
    scale/bias APs), so score matmuls contract 128 rows at full rate
    with FWL intact. (Measured: K=64 row-tiled pairs are ~16us SLOWER.)
  * PSUM discipline: 4 ring tags x 2 banks = all 8 banks. psA/psB hold
    QK projection psums then per-head score tiles; poA/poB hold V
    projection psums, then the attn@V accumulators [65,1024] (ones
    column of V_aug gives the softmax denominator), then the out-proj
    psums. Program order defines the ring handoffs.
  * exp on ACT is the roofline: 128 x [128,1024] Exp instructions
    (per-key mask bias AP, scale 1/8, no max subtraction needed since
    |scores/8| <= ~2). Everything else is kept off ACT: QK bias adds on
    DVE, attn-out normalization split across DVE and Pool.
"""

import numpy as np
import ml_dtypes
from contextlib import ExitStack

import concourse.tile as tile
from concourse import bacc, mybir
from concourse.bass_utils import run_bass_kernel_spmd

F32 = mybir.dt.float32
BF16 = mybir.dt.bfloat16
I32 = mybir.dt.int32
FP8 = mybir.dt.float8e4
AF = mybir.ActivationFunctionType

S = 2048
D = 1024
H_LOC = 4           # heads per core
DK = 64
DH = H_LOC * DK     # 256: d' per core
KT = D // 128       # 8 k-tiles for the D contraction
ST = S // 128       # 16 key tiles
INV_SCALE = 1.0 / 8.0

TRACE = False
TRACE_ALL_CORES = False
LAST_EXEC_NS = None
LAST_RESULTS = None
LAST_IN_MAPS = None

_CACHED_NC = None
FP8S = False


FE_A16 = 756387.69755859  # (2^23/ln2)/2 * INV_SCALE
FE_B1 = 1062775000.0      # calibrated: product-blend scale == 1, rms 0.9%
FE_P = 4194304            # 2^22: half-period phase shift via int add


FASTEXP = ()


def _build(phases=3, repeat=1, fastexp=None, fp8s=None):
    if fastexp is None:
        fastexp = FASTEXP
    if fp8s is None:
        fp8s = FP8S
    nc = bacc.Bacc("TRN2", target_bir_lowering=False, debug=False,
                   enable_asserts=True, num_devices=8)

    # All inputs are host-packed into the exact SBUF image (partition-major)
    # so every load is a contiguous identity DMA with large descriptors.
    xt_in = nc.dram_tensor("xt", [128, 4, KT, 512], BF16,
                           kind="ExternalInput").ap()
    w_q = nc.dram_tensor("w_q", [128, KT, DH], BF16, kind="ExternalInput").ap()
    w_k = nc.dram_tensor("w_k", [128, KT, DH], BF16, kind="ExternalInput").ap()
    w_v = nc.dram_tensor("w_v", [128, KT, DH], BF16, kind="ExternalInput").ap()
    w_out = nc.dram_tensor("w_out", [128, 2, D], BF16,
                           kind="ExternalInput").ap()
    b_q = nc.dram_tensor("b_q", [128, 2, 2], F32, kind="ExternalInput").ap()
    b_k = nc.dram_tensor("b_k", [128, 2, 2], F32, kind="ExternalInput").ap()
    sc01 = nc.dram_tensor("sc01", [128, 2], F32, kind="ExternalInput").ap()
    bv_bc_in = nc.dram_tensor("bv_bc", [128, DH], F32, kind="ExternalInput").ap()
    mask_bias = nc.dram_tensor("mask_bias", [128, ST], F32,
                               kind="ExternalInput").ap()

    y = nc.dram_tensor("y", [S, D], BF16, kind="ExternalOutput").ap()

    with tile.TileContext(nc) as tc, ExitStack() as ctx:
        if repeat > 1:
            ctx.enter_context(tc.For_i(0, repeat, 1))
        # ---------- persistent SBUF ----------
        persist = ctx.enter_context(tc.tile_pool(name="persist", bufs=1))

        xt_sb = persist.tile([128, 4, KT, 512], BF16, tag="xt")
        wq_sb = persist.tile([128, KT, DH], BF16, tag="wq")
        wk_sb = persist.tile([128, KT, DH], BF16, tag="wk")
        wv_sb = persist.tile([128, KT, DH], BF16, tag="wv")
        wo_sb = persist.tile([128, 2, D], BF16, tag="wo")
        # head 2m+a of plane m lives at partitions a*64:(a+1)*64 (unpadded)
        qk_dt = FP8 if fp8s else BF16
        qt = persist.tile([128, H_LOC, S], qk_dt, tag="qt")
        kt_sb = persist.tile([128, H_LOC, S], qk_dt, tag="kt")
        # flat per-kti V row: 4 heads x (64 V + ones + dead) + 64-col zero
        # tail so every head's 128-wide stationary window is initialized
        VROW = H_LOC * (DK + 2) + 64  # 328
        v_aug = persist.tile([128, ST, VROW], BF16, tag="vaug")
        out_ht = persist.tile([128, 2, S], BF16, tag="outht")
        bq_sb = persist.tile([128, 2, 2], F32, tag="bq")
        bk_sb = persist.tile([128, 2, 2], F32, tag="bk")
        sc01_sb = persist.tile([128, 2], F32, tag="sc01")
        bv_bc = persist.tile([128, DH], F32, tag="bvbc")
        mask_sb = persist.tile([128, ST], F32, tag="mask")

        # ones column of V_aug (gives softmax denominator via attn@V);
        # dead cols + tail zeroed so the 128-wide stationary windows
        # (see attn_v) never read uninitialized SBUF.
        for h in range(H_LOC):
            nc.vector.memset(v_aug[:, :, h * 66 + DK:h * 66 + DK + 1], 1.0)
            nc.vector.memset(v_aug[:, :, h * 66 + DK + 1:h * 66 + DK + 2],
                             0.0)
        nc.vector.memset(v_aug[:, :, 264:VROW], 0.0)
        # DMA: each engine queue drives its own ring at ~160GB/s, so the
        # bulk loads are spread over four rings, ordered by first use.
        # First matmul needs wk + xt qc0; qc0 is split k0-3/k4-7 across
        # two rings so it lands in ~3us instead of ~6.
        nc.sync.dma_start(xt_sb[:, 0, 0:KT // 2], xt_in[:, 0, 0:KT // 2])
        nc.gpsimd.dma_start(wk_sb[:], w_k)
        nc.gpsimd.dma_start(xt_sb[:, 0, KT // 2:], xt_in[:, 0, KT // 2:])
        nc.sync.dma_start(xt_sb[:, 1], xt_in[:, 1])
        nc.gpsimd.dma_start(xt_sb[:, 2], xt_in[:, 2])
        nc.sync.dma_start(xt_sb[:, 3], xt_in[:, 3])
        nc.scalar.dma_start(bk_sb[:], b_k)
        nc.scalar.dma_start(bq_sb[:], b_q)
        nc.scalar.dma_start(sc01_sb[:], sc01)
        nc.scalar.dma_start(mask_sb[:], mask_bias)
        nc.scalar.dma_start(wq_sb[:], w_q)
        nc.scalar.dma_start(wv_sb[:], w_v)
        nc.scalar.dma_start(bv_bc[:], bv_bc_in)
        nc.scalar.dma_start(wo_sb[:], w_out)

        # PSUM: "ps" tag is double-buffered (2 x 2 banks) so the next
        # score matmul never waits for the previous exp read (ring latency
        # was the measured critical path); poA/poB hold the attn@V
        # accumulators (2 banks each). Total = all 8 banks.
        pp = ctx.enter_context(tc.tile_pool(name="pp", bufs=1, space="PSUM"))

        def ps_tile(tag, shape=(128, 1024), name="pst"):
            return pp.tile(list(shape), F32, tag=tag, name=f"{name}_{tag}",
                           padded_shape=[128, 1024],
                           bufs=2 if tag == "ps" else 1)

        epool = ctx.enter_context(tc.tile_pool(name="expt", bufs=5))
        ipool = ctx.enter_context(tc.tile_pool(name="ipool", bufs=1))
        small = ctx.enter_context(tc.tile_pool(name="small", bufs=2))
        ocopy = ctx.enter_context(tc.tile_pool(name="ocopy", bufs=2))
        ypool = ctx.enter_context(tc.tile_pool(name="ypool", bufs=3))

        # ---------- emission helpers ----------
        def qk_half(wt, bt, dst, tagn, m, c, j):
            """Project 512 tokens (chunk c, half j) of Q or K, plane m."""
            p_q = ps_tile(tagn, (128, 512), name="pqk")
            for k in range(KT):
                nc.tensor.matmul(
                    p_q[:],
                    wt[:, k, m * 128:(m + 1) * 128],
                    xt_sb[:, 2 * c + j, k, :],
                    start=(k == 0), stop=(k == KT - 1))
            q0 = c * 1024 + j * 512
            for a in range(2):
                nc.vector.tensor_scalar(
                    out=dst[:, 2 * m + a, q0:q0 + 512], in0=p_q[:],
                    scalar1=sc01_sb[:, a:a + 1], scalar2=bt[:, m, a:a + 1],
                    op0=mybir.AluOpType.mult, op1=mybir.AluOpType.add)

        def qk_proj(wt, bt, dst, tagn, m, c):
            for j in range(2):
                qk_half(wt, bt, dst, tagn, m, c, j)

        def v_proj(st, tagn=None):
            if tagn is None:
                tagn = "poA" if st % 2 == 0 else "poB"
            p_v = ps_tile(tagn, (128, DH), name="pv")
            for k in range(KT):
                nc.tensor.matmul(
                    p_v[:],
                    xt_sb[:, st // 4, k, (st % 4) * 128:(st % 4) * 128 + 128],
                    wv_sb[:, k, :], start=(k == 0), stop=(k == KT - 1))
            nc.vector.tensor_add(
                v_aug[:, st, 0:264].rearrange(
                    "p (h d) -> p h d", h=H_LOC)[:, :, 0:DK],
                p_v[:].rearrange("p (h d) -> p h d", h=H_LOC),
                bv_bc[:].rearrange("p (h d) -> p h d", h=H_LOC))

        def scores(hm, qh, kti):
            """Row-tiled score pair (K=64 at base partitions 0/64, run
            concurrently by the PE) + the two exps. Returns exp tiles."""
            p_ss = [ps_tile("ps", name="ps") for a in range(2)]
            for j in range(2):
                for a in range(2):
                    h = 2 * hm + a
                    q0 = qh * 1024 + j * 512
                    if fp8s:
                        # fp8e4 DoubleRow at 0.5 cyc/row: ko dim is a
                        # stride-0 broadcast (each operand streamed twice,
                        # values pre-scaled by 1/2 -> psum = qk/2).
                        nc.tensor.matmul(
                            p_ss[a][:, j * 512:(j + 1) * 512],
                            kt_sb[:, h:h + 1, kti * 128:(kti + 1) * 128]
                            .to_broadcast([128, 2, 128]),
                            qt[:, h:h + 1, q0:q0 + 512]
                            .to_broadcast([128, 2, 512]),
                            start=True, stop=True,
                            perf_mode=mybir.MatmulPerfMode.DoubleRow)
                    else:
                        nc.tensor.matmul(
                            p_ss[a][:, j * 512:(j + 1) * 512],
                            kt_sb[:, h, kti * 128:(kti + 1) * 128],
                            qt[:, h, q0:q0 + 512], start=True, stop=True)
            e_ts = []
            for a in range(2):
                e_t = epool.tile([128, 1024], BF16, tag=f"et{a}",
                                 name=f"et{a}")
                if kti in fastexp:
                    # Schraudolph product-blend on DVE+Pool (mask==1 only):
                    # e^(s/8) ~ f(i1) * f(i1 + 2^22), calibrated to scale 1
                    # so it mixes with exact-exp tiles in the same softmax.
                    i1 = ipool.tile([128, 1024], I32, tag=f"i1{a}",
                                    name=f"i1{a}")
                    nc.vector.tensor_scalar(
                        out=i1[:], in0=p_ss[a][:], scalar1=FE_A16,
                        scalar2=FE_B1, op0=mybir.AluOpType.mult,
                        op1=mybir.AluOpType.add)
                    i2 = ipool.tile([128, 1024], I32, tag=f"i2{a}",
                                    name=f"i2{a}")
                    nc.vector.tensor_scalar_add(i2[:], i1[:], FE_P)
                    nc.gpsimd.tensor_mul(
                        e_t[:], i1[:].bitcast(F32), i2[:].bitcast(F32))
                else:
                    nc.scalar.activation(
                        e_t[:], p_ss[a][:], AF.Exp,
                        bias=mask_sb[:, kti:kti + 1],
                        scale=0.25 if fp8s else INV_SCALE)
                e_ts.append(e_t)
            return e_ts

        def attn_v(hm, po, e_ts, a, kti):
            # 128-col stationary (FWL-eligible: NumWeights==128): head h's
            # window runs into the next head's region (or the zero tail),
            # psum rows 0..63 = num, 64 = den, 65.. = ignored garbage.
            h = 2 * hm + a
            c0 = h * (DK + 2)
            for j in range(2):
                nc.tensor.matmul(
                    po[a][:, j * 512:(j + 1) * 512],
                    v_aug[:, kti, c0:c0 + 128],
                    e_ts[a][:, j * 512:(j + 1) * 512],
                    start=(kti == 0), stop=(kti == ST - 1),
                    skip_group_check=True)

        def normalize(hm, qh, po):
            # out_ht = po[base:base+64] * (1 / po[base+64]). The copy runs
            # FIRST so the po psum banks free after ~1.2us; the 6.5us DVE
            # reciprocal (8 cyc/elem, single lane) then runs off-ring from
            # the SBUF copy.
            for a in range(2):
                oc = ocopy.tile([DK + 1, 1024], F32, tag=f"oc{a}",
                                name=f"oc{a}")
                nc.vector.tensor_copy(oc[:], po[a][0:DK + 1, :])
                r_sb = small.tile([1, 1024], F32, tag=f"rsb{a}",
                                  name=f"rsb{a}")
                nc.vector.reciprocal(r_sb[0:1, :], oc[DK:DK + 1, :])
                bc_sb = small.tile([64, 1024], F32, tag=f"bcsb{a}",
                                   name=f"bcsb{a}")
                nc.gpsimd.partition_broadcast(
                    bc_sb[:], r_sb[0:1, :], channels=64)
                nc.vector.tensor_mul(
                    out_ht[a * 64:(a + 1) * 64, hm,
                           qh * 1024:(qh + 1) * 1024],
                    oc[0:DK, :], bc_sb[:])

        RINGS = ("poA", "poB", "ps", "ps")

        def out_proj(st, ring):
            p_y = ps_tile(RINGS[ring % 4], name="py")
            for k2 in range(2):
                for j in range(2):
                    nc.tensor.matmul(
                        p_y[:, j * 512:(j + 1) * 512],
                        out_ht[:, k2, st * 128:(st + 1) * 128],
                        wo_sb[:, k2, j * 512:(j + 1) * 512],
                        start=(k2 == 0), stop=(k2 == 1))
            y_sb = ypool.tile([128, D], BF16, tag="ysb", name="ysb")
            nc.vector.tensor_copy(out=y_sb[:], in_=p_y[:])
            nc.sync.dma_start(y[st * 128:(st + 1) * 128, :], y_sb[:])

        def attention(qh, hm, inject):
            """kti loop for one (q half, head pair); inject[kti] emits
            projection/out-proj work into this slot's PE slack."""
            po = [ps_tile("poA", (128, 1024), name="poa"),
                  ps_tile("poB", (128, 1024), name="pob")]
            e_prev = scores(hm, qh, 0)
            for kti in range(ST):
                e_cur = e_prev
                # scores(kti+1) FIRST: the PE queue is in-order, so the
                # exp(kti)->scores(kti+1)->exp(kti+1) ring cycle must not
                # have the attn@V matmuls in the middle of it.
                if kti + 1 < ST:
                    e_prev = scores(hm, qh, kti + 1)
                attn_v(hm, po, e_cur, 0, kti)
                for fn in inject.get(kti, ()):
                    fn()
                attn_v(hm, po, e_cur, 1, kti)
            normalize(hm, qh, po)

        # ---------- pipelined emission ----------
        if phases >= 1:
            # minimal pre-loop: scores(qh0,hm0,kti 0..3) need only
            # K(m0,c0,j0) and Q(m0,c0); V(0,1) covers the first attn@Vs.
            qk_half(wk_sb, bk_sb, kt_sb, "ps", 0, 0, 0)
            qk_proj(wq_sb, bq_sb, qt, "ps", 0, 0)
            v_proj(0)
            v_proj(1)
        if phases >= 2:
            # Injected work inside an attention loop rides the short-lived
            # psA/psB score rings (poA/poB are held by the accumulators).
            # V(2..15), K(m1), Q(m1 c0) ride the (qh0, hm0) slots;
            # Q(m0 c1), Q(m1 c1) ride the (qh0, hm1) slots.
            # injection map: rest of K(m0) early (scores need kt[kti//4]
            # by slot kti), V(st) strictly before slot st, K(m1)/Q(m1,c0)
            # late (consumed by the hm=1 loop).
            KH, QH_ = (lambda *args: lambda: qk_half(wk_sb, bk_sb, kt_sb,
                                                     "ps", *args),
                       lambda *args: lambda: qk_half(wq_sb, bq_sb, qt,
                                                     "ps", *args))
            VP = lambda s: lambda: v_proj(s, "ps")
            inj00 = {0: [KH(0, 0, 1)], 1: [VP(2), VP(3)],
                     2: [KH(0, 1, 0)], 3: [VP(4), VP(5)],
                     4: [KH(0, 1, 1)], 5: [VP(6), VP(7)],
                     6: [VP(8), VP(9)], 7: [VP(10), VP(11)],
                     8: [VP(12), VP(13)], 9: [VP(14), VP(15)],
                     10: [KH(1, 0, 0)], 11: [KH(1, 0, 1)],
                     12: [QH_(1, 0, 0)], 13: [QH_(1, 0, 1)],
                     14: [KH(1, 1, 0)], 15: [KH(1, 1, 1)]}
            attention(0, 0, inj00)
            inj01 = {2: [QH_(0, 1, 0)], 4: [QH_(0, 1, 1)],
                     8: [QH_(1, 1, 0)], 10: [QH_(1, 1, 1)]}
            attention(0, 1, inj01)
            # qh boundary: two qh0 out-proj tiles clear through the po rings
            # before (qh1, hm0) starts accumulating; the remaining six ride
            # the score rings inside the (qh1, hm0) loop.
            inj10 = {}
            if phases >= 3:
                out_proj(0, 0)
                out_proj(1, 1)
                inj10 = {k: [lambda s=2 + k: out_proj(s, 2)]
                         for k in range(6)}
            attention(1, 0, inj10)
            attention(1, 1, {})
            if phases >= 3:
                for sti in range(8):
                    out_proj(8 + sti, sti)

    nc.compile()
    return nc


def kernel(x, mask, W_qkv, b_qkv, W_out, b_out):
    global _CACHED_NC, LAST_EXEC_NS, LAST_RESULTS, LAST_IN_MAPS
    x = np.asarray(x, dtype=np.float32)
    mask = np.asarray(mask)
    W_qkv = np.asarray(W_qkv, dtype=np.float32)
    b_qkv = np.asarray(b_qkv, dtype=np.float32)
    W_out = np.asarray(W_out, dtype=np.float32)
    b_out_full = np.asarray(b_out, dtype=np.float32)

    B = x.shape[0]
    if _CACHED_NC is None:
        _CACHED_NC = _build(fp8s=FP8S)
    nc = _CACHED_NC
    alpha = 0.5 if FP8S else 1.0

    bf = ml_dtypes.bfloat16

    def pack_w(w):  # [D, dh] -> [128, KT, dh] partition-major image
        return np.ascontiguousarray(
            w.reshape(KT, 128, -1).transpose(1, 0, 2)).astype(bf)

    def pack_vec(v):  # [(m*128+p)] -> [128, m]
        return np.ascontiguousarray(v.reshape(-1, 128).T)

    def pack_bias(v):  # [128, m, a]: bias on head a's home rows, else 0
        bm = v.reshape(2, 128).T           # [128, m]
        out = np.zeros((128, 2, 2), np.float32)
        out[0:64, :, 0] = bm[0:64]
        out[64:128, :, 1] = bm[64:128]
        return out

    sc01 = np.zeros((128, 2), np.float32)
    sc01[0:64, 0] = alpha
    sc01[64:128, 1] = alpha

    mask_bias = ((mask.astype(np.float32) - 1.0) * 1e9).astype(np.float32)
    # x^T image: (p, qc, t, s') = x[b][qc*512+s', t*128+p]
    xts = [np.ascontiguousarray(
        x[b].reshape(4, 512, KT, 128).transpose(3, 0, 2, 1)).astype(bf)
        for b in range(B)]

    in_maps = []
    for c in range(8):
        b = c // 4
        g = c % 4
        cs = g * DH
        in_maps.append({
            "xt": xts[b],
            "bv_bc": np.broadcast_to(
                b_qkv[2 * D + cs:2 * D + cs + DH], (128, DH)).copy(),
            "w_q": pack_w(W_qkv[:, cs:cs + DH]),
            "w_k": pack_w(W_qkv[:, D + cs:D + cs + DH]),
            "w_v": pack_w(W_qkv[:, 2 * D + cs:2 * D + cs + DH]),
            "b_q": pack_bias(b_qkv[cs:cs + DH]) * alpha,
            "b_k": pack_bias(b_qkv[D + cs:D + cs + DH]) * alpha,
            "sc01": sc01,
            "w_out": np.ascontiguousarray(
                W_out[cs:cs + DH, :].reshape(2, 128, D)
                .transpose(1, 0, 2)).astype(bf),
            "mask_bias": pack_vec(mask_bias[b]),
        })

    kwargs = {}
    if TRACE:
        kwargs["trace"] = True
        if TRACE_ALL_CORES:
            kwargs["trace_cores"] = list(range(8))
    LAST_IN_MAPS = in_maps
    res = None
    for attempt in range(3):
        try:
            res = run_bass_kernel_spmd(nc, in_maps, core_ids=list(range(8)),
                                       **kwargs)
            break
        except Exception:
            if attempt == 2:
                raise
            import time as _time
            _time.sleep(2.0)
    LAST_EXEC_NS = res.exec_time_ns
    LAST_RESULTS = res

    out = np.zeros((B, S, D), dtype=np.float32)
    for c in range(8):
        out[c // 4] += res.results[c]["y"].astype(np.float32)
    out += b_out_full
    return out

